# revision 16
# baseline (speedup 1.0000x reference)
"""Trainium2 Bass kernel for nn_Encoder_81595788689580.

Attention-gated GRU encoder: per time step
    w1 = h @ attn1_W.T + attn1_b
    w2 = x_t @ attn2_W.T + attn2_b
    v  = tanh(w1 + w2) @ attn3_W.T + attn3_b
    alpha = softmax(v, axis=feature)
    wx = x_t * alpha
    GRU cell (r, z, n) -> h_new
Output: [B, T, H] hidden states.

Strategy (8 NeuronCores, data-parallel over batch):
  - batch 4096 -> 512 rows per core; all weights replicated; everything
    transposed on chip (features on partitions, batch on the free dim).
  - all five weight matmuls run in fp8 e4m3 with DoubleRow perf mode:
    2 K-planes (256 contraction rows) per instruction at 0.5 PE
    cycles/output-column -- ~4x the fp16 matmul rate.
  - odd K-plane counts (I=320 -> 2.5 planes) use a zero-pair trick: the
    last DR instruction pairs moving planes (1,2) against a stationary
    pair (0, W_2), so no pad plane is ever materialized in SBUF.
  - numerics (sim rel err 9.2e-3 vs 2e-2 budget):
      * weights, x, h, u, wx quantized to e4m3; ev and the GRU tail fp16
      * gate psums carry a x256 scale (wx pre-scaled by 256/den, W_hh
        stored x256) so wx lands in fp8's normal range; undone via ACT
        scale params (1/512 for r,z; 1/256 for n)
      * W_hh additionally gets a quantization-residual matmul
        (Wh ~ Wh8 + Rh8), killing the dominant fp8 error term
  - softmax denominator via a fp16 ones-matmul (value 1/256, so the
    reciprocal directly yields the 256-scaled normalizer); softmax max-
    subtraction skipped (attn3_b carries a -2 shift for range).
  - sigmoid computed as 0.5*tanh(x/2)+0.5 so every ACT op stays on the
    exp_and_others table (no table swaps).
  - the 512-row batch runs as 2 chunks of 256 so the two recurrences
    pipeline against each other; fp8 requantization (wx, h) runs on the
    otherwise-idle GPSIMD engine to unload DVE.
"""

import numpy as np

B, T, I, H = 4096, 24, 320, 256
NCORES = 8
BS = B // NCORES          # 512 rows per core
IP = 384                  # I padded to 3x128
KI = 3                    # feature planes
KH = 2                    # hidden planes
G = 3 * H                 # 768 gate rows
NCHUNK = 2
CB = BS // NCHUNK         # 256 batch columns per chunk

_STATE = {}
DEBUG_DUMP = False


def _build(t_steps=T):
    import concourse.bass as bass
    import concourse.tile as tile
    from concourse import bacc, mybir

    f32 = mybir.dt.float32
    F16 = mybir.dt.float16
    F8 = mybir.dt.float8e4
    AF = mybir.ActivationFunctionType
    OP = mybir.AluOpType
    DR = mybir.MatmulPerfMode.DoubleRow

    nc = bacc.Bacc("TRN2", target_bir_lowering=False, debug=False,
                   num_devices=NCORES)

    xT8 = nc.dram_tensor("xT8", [t_steps, 128, KI, BS], F8,
                         kind="ExternalInput").ap()
    xT16 = nc.dram_tensor("xT16", [t_steps, 128, KI, BS], F16,
                          kind="ExternalInput").ap()
    h0T8 = nc.dram_tensor("h0T8", [128, KH, BS], F8, kind="ExternalInput").ap()
    h0T16 = nc.dram_tensor("h0T16", [128, KH, BS], F16,
                           kind="ExternalInput").ap()
    wat1 = nc.dram_tensor("wat1", [128, 2, IP], F8, kind="ExternalInput").ap()
    wat2 = nc.dram_tensor("wat2", [128, 4, IP], F8, kind="ExternalInput").ap()
    wat3 = nc.dram_tensor("wat3", [128, 4, IP], F8, kind="ExternalInput").ap()
    wih = nc.dram_tensor("wih", [128, 4, G], F8, kind="ExternalInput").ap()
    whh = nc.dram_tensor("whh", [128, 2, G], F8, kind="ExternalInput").ap()
    rhh = nc.dram_tensor("rhh", [128, 2, G], F8, kind="ExternalInput").ap()
    onesw = nc.dram_tensor("onesw", [128, 128], F16, kind="ExternalInput").ap()
    bias_u_d = nc.dram_tensor("bias_u", [128, KI], f32, kind="ExternalInput").ap()
    bias_v_d = nc.dram_tensor("bias_v", [128, KI], f32, kind="ExternalInput").ap()
    bias_rzh_d = nc.dram_tensor("bias_rzh", [128, 4], f32,
                                kind="ExternalInput").ap()
    bias_hn_d = nc.dram_tensor("bias_hn", [128, 2], f32, kind="ExternalInput").ap()
    bias_in_d = nc.dram_tensor("bias_in", [128, 2], f32, kind="ExternalInput").ap()
    outT = nc.dram_tensor("outT", [t_steps, 128, KH, BS], F16,
                          kind="ExternalOutput").ap()
    dbg = {}
    if DEBUG_DUMP:
        for nm, shp, dt_ in [
                ("d_psu", [128, KI, CB], f32), ("d_u8", [128, KI, CB], F8),
                ("d_psv", [128, KI, CB], f32), ("d_ev", [128, KI, CB], F16),
                ("d_den", [128, CB], f32), ("d_rinv", [128, CB], F16),
                ("d_wxt", [128, KI, CB], F16), ("d_wx8", [128, KI, CB], F8),
                ("d_psr", [128, 2, CB], f32), ("d_psz", [128, 2, CB], f32),
                ("d_pshn", [128, 2, CB], f32), ("d_psin", [128, 2, CB], f32),
                ("d_g", [128, 4, CB], F16), ("d_t1h", [128, 2, CB], F16),
                ("d_p", [128, 2, CB], F16), ("d_n", [128, 2, CB], F16),
                ("d_h8", [128, KH, CB], F8)]:
            dbg[nm] = nc.dram_tensor(nm, shp, dt_, kind="ExternalOutput").ap()

    with tile.TileContext(nc) as tc:
        with tc.tile_pool(name="const", bufs=1) as cp, \
             tc.tile_pool(name="xs", bufs=1) as xp, \
             tc.tile_pool(name="hs", bufs=1) as hp, \
             tc.tile_pool(name="wk", bufs=1) as wp, \
             tc.tile_pool(name="ps", bufs=1, space="PSUM") as pp:

            w1t = cp.tile([128, 2, IP], F8)
            w2t = cp.tile([128, 4, IP], F8)
            w3t = cp.tile([128, 4, IP], F8)
            wiht = cp.tile([128, 4, G], F8)
            whht = cp.tile([128, 2, G], F8)
            rhht = cp.tile([128, 2, G], F8)
            onest = cp.tile([128, 128], F16)
            bu = cp.tile([128, KI], f32)
            bv = cp.tile([128, KI], f32)
            brzh = cp.tile([128, 4], f32)
            bhn = cp.tile([128, 2], f32)
            bin_ = cp.tile([128, 2], f32)

            # h0 + step-0 x first (they gate the first matmuls), then
            # weights ordered by first use, alternating the two HWDGE rings
            hcur8 = []
            hcur16 = []
            for ci in range(NCHUNK):
                cs = slice(ci * CB, (ci + 1) * CB)
                h8 = hp.tile([128, KH, CB], F8, name=f"h8_{ci}",
                             tag=f"h8{ci}", bufs=2)
                nc.scalar.dma_start(out=h8[:], in_=h0T8[:, :, cs])
                h16 = hp.tile([128, KH, CB], F16, name=f"h16_{ci}",
                              tag=f"h16{ci}", bufs=2)
                nc.scalar.dma_start(out=h16[:], in_=h0T16[:, :, cs])
                hcur8.append(h8)
                hcur16.append(h16)
            x8_pre = xp.tile([128, KI, BS], F8, name="x8_pre", tag="x8", bufs=3)
            nc.sync.dma_start(out=x8_pre[:], in_=xT8[0])
            x16_pre = xp.tile([128, KI, BS], F16, name="x16_pre", tag="x16",
                              bufs=3)
            nc.sync.dma_start(out=x16_pre[:], in_=xT16[0])
            for i, (dst, src) in enumerate([
                    (w2t, wat2), (w1t, wat1), (bu, bias_u_d),
                    (w3t, wat3), (bv, bias_v_d), (onest, onesw),
                    (whht, whh), (rhht, rhh), (wiht, wih),
                    (brzh, bias_rzh_d), (bhn, bias_hn_d),
                    (bin_, bias_in_d)]):
                eng = nc.sync if i % 2 == 0 else nc.scalar
                eng.dma_start(out=dst[:], in_=src)

            def ms(m):
                return slice(m * 128, (m + 1) * 128)

            def mm(out, w, pair, mov, start, stop):
                nc.tensor.matmul(out, w[:, 2 * pair:2 * pair + 2, :],
                                 mov, start=start, stop=stop, perf_mode=DR)

            for t in range(t_steps):
                if t == 0:
                    x8_t, x16_t = x8_pre, x16_pre
                else:
                    x8_t = xp.tile([128, KI, BS], F8, name=f"x8_{t}",
                                   tag="x8", bufs=3)
                    nc.sync.dma_start(out=x8_t[:], in_=xT8[t])
                    x16_t = xp.tile([128, KI, BS], F16, name=f"x16_{t}",
                                    tag="x16", bufs=3)
                    nc.sync.dma_start(out=x16_t[:], in_=xT16[t])

                st = [{} for _ in range(NCHUNK)]

                # ---- phase 1: attention stage 1 (u = tanh(w1+w2)) ----
                # chunk-interleaved matmuls: consecutive instructions share
                # the same stationary so weight loads can be deduped/hidden
                ps_u = [pp.tile([128, KI, CB], f32,
                                name=f"psu_{t}_{ci}", tag="aps",
                                bufs=2) for ci in range(NCHUNK)]
                for m in range(KI):
                    for ci in range(NCHUNK):
                        cs = slice(ci * CB, (ci + 1) * CB)
                        mm(ps_u[ci][:, m, :], w2t[:, :, ms(m)], 0,
                           x8_t[:, 0:2, cs], True, False)
                    for ci in range(NCHUNK):
                        cs = slice(ci * CB, (ci + 1) * CB)
                        mm(ps_u[ci][:, m, :], w2t[:, :, ms(m)], 1,
                           x8_t[:, 1:3, cs], False, False)
                    for ci in range(NCHUNK):
                        mm(ps_u[ci][:, m, :], w1t[:, :, ms(m)], 0,
                           hcur8[ci][:, 0:2, :], False, True)
                for ci in range(NCHUNK):
                    u8 = wp.tile([128, KI, CB], F8, name=f"u_{t}_{ci}",
                                 tag="u", bufs=3)
                    for m in range(KI):
                        nc.scalar.activation(u8[:, m, :], ps_u[ci][:, m, :],
                                             AF.Tanh, bias=bu[:, m:m + 1])
                    if dbg and t == 0 and ci == 0:
                        dcp = wp.tile([128, KI, CB], f32, name="dcp_u")
                        nc.vector.tensor_copy(dcp[:], ps_u[0][:])
                        nc.scalar.dma_start(out=dbg["d_psu"], in_=dcp[:])
                        nc.scalar.dma_start(out=dbg["d_u8"], in_=u8[:])
                    st[ci].update(u=u8)

                # ---- phase 2: v, softmax, wx ----
                ps_v = [pp.tile([128, KI, CB], f32,
                                name=f"psv_{t}_{ci}", tag="aps",
                                bufs=2) for ci in range(NCHUNK)]
                for m in range(KI):
                    for ci in range(NCHUNK):
                        mm(ps_v[ci][:, m, :], w3t[:, :, ms(m)], 0,
                           st[ci]["u"][:, 0:2, :], True, False)
                    for ci in range(NCHUNK):
                        mm(ps_v[ci][:, m, :], w3t[:, :, ms(m)], 1,
                           st[ci]["u"][:, 1:3, :], False, True)
                for ci in range(NCHUNK):
                    cs = slice(ci * CB, (ci + 1) * CB)
                    ev = wp.tile([128, KI, CB], F16, name=f"ev_{t}_{ci}",
                                 tag="ev", bufs=3)
                    for m in range(KI):
                        nc.scalar.activation(ev[:, m, :], ps_v[ci][:, m, :],
                                             AF.Exp, bias=bv[:, m:m + 1])
                    ps_den = pp.tile([128, CB], f32, name=f"psden_{t}_{ci}",
                                     tag="aps", bufs=2)
                    for k in range(KI):
                        nc.tensor.matmul(ps_den[:], onest[:], ev[:, k, :],
                                         start=(k == 0), stop=(k == KI - 1))
                    rinv = wp.tile([128, CB], f32, name=f"rinv_{t}_{ci}",
                                   tag="rinv", bufs=3)
                    nc.vector.reciprocal_approx_fast(rinv[:], ps_den[:])
                    rinv16 = wp.tile([128, CB], F16, name=f"rinv16_{t}_{ci}",
                                     tag="rinv16", bufs=3)
                    nc.vector.tensor_copy(rinv16[:], rinv[:])
                    wxt = wp.tile([128, KI, CB], F16, name=f"wxt_{t}_{ci}",
                                  tag="wxt", bufs=3)
                    nc.vector.tensor_mul(wxt[:], x16_t[:, :, cs], ev[:])
                    wx8 = wp.tile([128, KI, CB], F8, name=f"wx_{t}_{ci}",
                                  tag="wx", bufs=3)
                    _r = rinv16[:]
                    rrep = bass.AP(tensor=_r.tensor, offset=_r.offset,
                                   ap=[_r.ap[0], [0, KI], _r.ap[1]])
                    nc.vector.tensor_mul(wx8[:], wxt[:], rrep)
                    if dbg and t == 0 and ci == 0:
                        dcpv = wp.tile([128, KI, CB], f32, name="dcpv")
                        nc.vector.tensor_copy(dcpv[:], ps_v[0][:])
                        nc.scalar.dma_start(out=dbg["d_psv"], in_=dcpv[:])
                        nc.scalar.dma_start(out=dbg["d_ev"], in_=ev[:])
                        dcpd = wp.tile([128, CB], f32, name="dcpd")
                        nc.vector.tensor_copy(dcpd[:], ps_den[:])
                        nc.scalar.dma_start(out=dbg["d_den"], in_=dcpd[:])
                        nc.scalar.dma_start(out=dbg["d_rinv"], in_=rinv16[:])
                        nc.scalar.dma_start(out=dbg["d_wxt"], in_=wxt[:])
                        nc.scalar.dma_start(out=dbg["d_wx8"], in_=wx8[:])
                    st[ci].update(wx=wx8)

                # ---- phase 3: gate matmuls + GRU tail (per chunk so the
                # two recurrences pipeline; gate lds not shared) ----
                ps_hn = [None] * NCHUNK
                ps_r = [None] * NCHUNK
                ps_z = [None] * NCHUNK
                ps_in = [None] * NCHUNK
                for ci in range(NCHUNK):
                    h8 = hcur8[ci]
                    wx8 = st[ci]["wx"]
                    ps_hn[ci] = pp.tile([128, 2, CB], f32,
                                        name=f"pshn_{t}_{ci}", tag="gps",
                                        bufs=4)
                    for m in range(2):
                        mm(ps_hn[ci][:, m, :], whht[:, :, ms(4 + m)], 0,
                           h8[:, 0:2, :], True, False)
                        mm(ps_hn[ci][:, m, :], rhht[:, :, ms(4 + m)], 0,
                           h8[:, 0:2, :], False, True)
                    ps_r[ci] = pp.tile([128, 2, CB], f32,
                                       name=f"psr_{t}_{ci}", tag="gps",
                                       bufs=4)
                    ps_z[ci] = pp.tile([128, 2, CB], f32,
                                       name=f"psz_{t}_{ci}", tag="gps",
                                       bufs=4)
                    # h-only matmuls of the m0 slices first: they keep the
                    # PE fed while wx lands. NOTE open accumulation groups
                    # must sit in DIFFERENT psum banks: r/z are separate
                    # banks, but m0/m1 of one tile share a bank, so m1 only
                    # opens after m0 closes.
                    for mm_t, base in ((ps_r[ci], 0), (ps_z[ci], 2)):
                        mm(mm_t[:, 0, :], whht[:, :, ms(base)], 0,
                           h8[:, 0:2, :], True, False)
                        mm(mm_t[:, 0, :], rhht[:, :, ms(base)], 0,
                           h8[:, 0:2, :], False, False)
                    for mm_t, base in ((ps_r[ci], 0), (ps_z[ci], 2)):
                        mm(mm_t[:, 0, :], wiht[:, :, ms(base)], 0,
                           wx8[:, 0:2, :], False, False)
                        mm(mm_t[:, 0, :], wiht[:, :, ms(base)], 1,
                           wx8[:, 1:3, :], False, True)
                        mm(mm_t[:, 1, :], whht[:, :, ms(base + 1)], 0,
                           h8[:, 0:2, :], True, False)
                        mm(mm_t[:, 1, :], rhht[:, :, ms(base + 1)], 0,
                           h8[:, 0:2, :], False, False)
                        mm(mm_t[:, 1, :], wiht[:, :, ms(base + 1)], 0,
                           wx8[:, 0:2, :], False, False)
                        mm(mm_t[:, 1, :], wiht[:, :, ms(base + 1)], 1,
                           wx8[:, 1:3, :], False, True)
                    ps_in[ci] = pp.tile([128, 2, CB], f32,
                                        name=f"psin_{t}_{ci}", tag="gps",
                                        bufs=4)
                    for m in range(2):
                        mm(ps_in[ci][:, m, :], wiht[:, :, ms(4 + m)], 0,
                           wx8[:, 0:2, :], True, False)
                        mm(ps_in[ci][:, m, :], wiht[:, :, ms(4 + m)], 1,
                           wx8[:, 1:3, :], False, True)

                for ci in range(NCHUNK):
                    cs = slice(ci * CB, (ci + 1) * CB)
                    h16 = hcur16[ci]
                    if dbg and t == 0 and ci == 0:
                        for dnm, pst in [("d_psr", ps_r[0]), ("d_psz", ps_z[0]),
                                         ("d_pshn", ps_hn[0]),
                                         ("d_psin", ps_in[0])]:
                            dcpg = wp.tile([128, 2, CB], f32, name=f"dc{dnm}")
                            nc.vector.tensor_copy(dcpg[:], pst[:])
                            nc.scalar.dma_start(out=dbg[dnm], in_=dcpg[:])
                    g = wp.tile([128, 4, CB], F16, name=f"g_{t}_{ci}",
                                tag="g", bufs=3)
                    for m in range(4):
                        src_ps = ps_r[ci] if m < 2 else ps_z[ci]
                        nc.scalar.activation(g[:, m, :], src_ps[:, m % 2, :],
                                             AF.Tanh, bias=brzh[:, m:m + 1],
                                             scale=1.0 / 512.0)
                    t1h = wp.tile([128, 2, CB], F16, name=f"t1h_{t}_{ci}",
                                  tag="t1h", bufs=3)
                    for m in range(2):
                        nc.vector.tensor_scalar(
                            out=t1h[:, m, :], in0=ps_hn[ci][:, m, :],
                            scalar1=bhn[:, m:m + 1], scalar2=0.5,
                            op0=OP.add, op1=OP.mult)
                    # p = (i_n' + 256 b_in) + t1h' is g-independent: compute
                    # it early so only two fp16 DVE ops trail the gate ACT
                    p_ = wp.tile([128, 2, CB], F16, name=f"p_{t}_{ci}",
                                 tag="p", bufs=3)
                    for m in range(2):
                        nc.vector.scalar_tensor_tensor(
                            p_[:, m, :], ps_in[ci][:, m, :], bin_[:, m:m + 1],
                            t1h[:, m, :], OP.add, OP.add)
                    t0h = wp.tile([128, 2, CB], F16, name=f"t0h_{t}_{ci}",
                                  tag="t0h", bufs=3)
                    nc.vector.tensor_mul(t0h[:], t1h[:], g[:, 0:2, :])
                    s2 = wp.tile([128, 2, CB], F16, name=f"s2_{t}_{ci}",
                                 tag="s2", bufs=3)
                    nc.vector.tensor_add(s2[:], t0h[:], p_[:])
                    n = wp.tile([128, 2, CB], F16, name=f"n_{t}_{ci}",
                                tag="n", bufs=3)
                    nc.scalar.activation(n[:], s2[:], AF.Tanh,
                                         scale=1.0 / 256.0)

                    zz = wp.tile([128, 2, CB], F16, name=f"zz_{t}_{ci}",
                                 tag="zz", bufs=3)
                    nc.vector.tensor_scalar(
                        out=zz[:], in0=g[:, 2:4, :], scalar1=0.5, scalar2=0.5,
                        op0=OP.mult, op1=OP.add)
                    w1z = wp.tile([128, 2, CB], F16, name=f"w1z_{t}_{ci}",
                                  tag="w1z", bufs=3)
                    nc.vector.tensor_scalar(
                        out=w1z[:], in0=g[:, 2:4, :], scalar1=-0.5,
                        scalar2=0.5, op0=OP.mult, op1=OP.add)
                    bzh = wp.tile([128, 2, CB], F16, name=f"bzh_{t}_{ci}",
                                  tag="bzh", bufs=3)
                    nc.vector.tensor_mul(bzh[:], zz[:], h16[:])
                    a4 = wp.tile([128, 2, CB], F16, name=f"a4_{t}_{ci}",
                                 tag="a4", bufs=3)
                    nc.vector.tensor_mul(a4[:], w1z[:], n[:])
                    h16n = hp.tile([128, KH, CB], F16, name=f"hn16_{t}_{ci}",
                                   tag=f"h16{ci}", bufs=2)
                    nc.vector.tensor_add(h16n[:], a4[:], bzh[:])
                    h8n = hp.tile([128, KH, CB], F8, name=f"hn8_{t}_{ci}",
                                  tag=f"h8{ci}", bufs=2)
                    nc.vector.tensor_copy(h8n[:], h16n[:])
                    if dbg and t == 0 and ci == 0:
                        nc.scalar.dma_start(out=dbg["d_g"], in_=g[:])
                        nc.scalar.dma_start(out=dbg["d_t1h"], in_=t1h[:])
                        nc.scalar.dma_start(out=dbg["d_p"], in_=p_[:])
                        nc.scalar.dma_start(out=dbg["d_n"], in_=n[:])
                        nc.scalar.dma_start(out=dbg["d_h8"], in_=h8n[:])
                    hcur16[ci] = h16n
                    hcur8[ci] = h8n

                    nc.sync.dma_start(out=outT[t][:, :, cs], in_=h16n[:])

    nc.compile()
    return nc


# ---------------- host-side data prep ----------------

def _np8():
    from concourse import mybir
    return mybir.dt.np(mybir.dt.float8e4)


def _q8(a, np8):
    return np.clip(np.asarray(a, np.float32), -240, 240).astype(np8)


def _prep_core_inputs(x, h0, attn1_W, attn1_b, attn2_W, attn2_b, attn3_W,
                      attn3_b, W_ih, b_ih, W_hh, b_hh, t_steps=T):
    f4 = np.float32
    np8 = _np8()
    x = np.asarray(x, f4)
    h0 = np.asarray(h0, f4)

    def zero_pair_KI(lhsT, out_cols):
        # lhsT [IP, out_cols] (rows = in-features zero-padded to 384)
        # -> [128, 4, out_cols] with plane order (0, 1, zero, 2)
        planes = lhsT.reshape(KI, 128, out_cols)
        stk = np.zeros((4, 128, out_cols), f4)
        stk[0], stk[1], stk[3] = planes[0], planes[1], planes[2]
        return _q8(stk.transpose(1, 0, 2), np8).copy()

    A1 = np.asarray(attn1_W, f4)                       # [I, H]
    w1 = np.zeros((H, IP), f4)
    w1[:, :I] = A1.T                                   # lhsT[hh, ii]
    wat1 = _q8(np.ascontiguousarray(
        w1.reshape(KH, 128, IP).transpose(1, 0, 2)), np8)

    A2 = np.asarray(attn2_W, f4)                       # [I, I] (out, in)
    w2 = np.zeros((IP, IP), f4)
    w2[:I, :I] = A2.T                                  # lhsT[in, out]
    wat2 = zero_pair_KI(w2, IP)

    A3 = np.asarray(attn3_W, f4)
    w3 = np.zeros((IP, IP), f4)
    w3[:I, :I] = A3.T
    wat3 = zero_pair_KI(w3, IP)

    Wi = np.asarray(W_ih, f4)                          # [G, I]
    wi = np.zeros((IP, G), f4)
    wi[:I, :] = Wi.T
    wih = zero_pair_KI(wi, G)

    Wh = np.asarray(W_hh, f4)                          # [G, H]
    wh_t = Wh.T * 256.0                                # [H, G], x256 scale
    wh8 = _q8(wh_t, np8)
    rh8 = _q8(wh_t - wh8.astype(f4), np8)              # quantization residual
    whh = np.ascontiguousarray(
        wh8.reshape(KH, 128, G).transpose(1, 0, 2))
    rhh = np.ascontiguousarray(
        rh8.reshape(KH, 128, G).transpose(1, 0, 2))

    onesw = np.full((128, 128), 1.0 / 256.0, np.float16)

    bu_v = np.zeros(IP, f4)
    bu_v[:I] = np.asarray(attn1_b, f4) + np.asarray(attn2_b, f4)
    bias_u = np.ascontiguousarray(bu_v.reshape(KI, 128).T)
    bvv = np.full(IP, -1e4, f4)
    bvv[:I] = np.asarray(attn3_b, f4) - 2.0   # shift-invariant, fp16 range
    bias_v = np.ascontiguousarray(bvv.reshape(KI, 128).T)
    brz = (np.asarray(b_ih, f4) + np.asarray(b_hh, f4))[:2 * H] * 0.5
    bias_rzh = np.ascontiguousarray(brz.reshape(4, 128).T)
    bias_hn = np.ascontiguousarray(
        (np.asarray(b_hh, f4)[2 * H:] * 256.0).reshape(2, 128).T)
    bias_in = np.ascontiguousarray(
        (np.asarray(b_ih, f4)[2 * H:] * 256.0).reshape(2, 128).T)

    xs = x[:, :t_steps, :]
    xpad16 = np.pad(xs.astype(np.float16), ((0, 0), (0, 0), (0, IP - I)))
    xpad8 = np.pad(_q8(xs, np8), ((0, 0), (0, 0), (0, IP - I)))
    # [NC, BS, T, KI, 128] -> [NC, T, 128, KI, BS]
    xr16 = xpad16.reshape(NCORES, BS, t_steps, KI, 128).transpose(0, 2, 4, 3, 1)
    xr8 = xpad8.reshape(NCORES, BS, t_steps, KI, 128).transpose(0, 2, 4, 3, 1)
    h0r16 = h0.astype(np.float16).reshape(NCORES, BS, KH, 128).transpose(0, 3, 2, 1)
    h0r8 = _q8(h0, np8).reshape(NCORES, BS, KH, 128).transpose(0, 3, 2, 1)

    shared = dict(wat1=wat1, wat2=wat2, wat3=wat3, wih=wih, whh=whh, rhh=rhh,
                  onesw=onesw, bias_u=bias_u, bias_v=bias_v,
                  bias_rzh=bias_rzh, bias_hn=bias_hn, bias_in=bias_in)
    in_maps = []
    for c in range(NCORES):
        m = dict(shared)
        m["xT8"] = np.ascontiguousarray(xr8[c])
        m["xT16"] = np.ascontiguousarray(xr16[c])
        m["h0T8"] = np.ascontiguousarray(h0r8[c])
        m["h0T16"] = np.ascontiguousarray(h0r16[c])
        in_maps.append(m)
    return in_maps


def _gather(results, t_steps=T):
    outs = []
    for c in range(NCORES):
        o = np.asarray(results[c]["outT"], np.float32)
        outs.append(o.transpose(3, 0, 2, 1).reshape(BS, t_steps, H))
    return np.ascontiguousarray(np.concatenate(outs, axis=0))


def _get_nc(t_steps=T):
    key = ("nc", t_steps)
    if key not in _STATE:
        _STATE[key] = _build(t_steps)
    return _STATE[key]


def run(inputs, trace=False, t_steps=T):
    from concourse.bass_utils import run_bass_kernel_spmd
    nc = _get_nc(t_steps)
    in_maps = _prep_core_inputs(t_steps=t_steps, **inputs)
    res = run_bass_kernel_spmd(nc, in_maps, list(range(NCORES)), trace=trace)
    return _gather(res.results, t_steps), res


def kernel(**inputs):
    out, _ = run(inputs, trace=False)
    return out


# revision 18
# speedup vs baseline: 1.2367x; 1.2367x over previous
"""Trainium2 Bass kernel for nn_Encoder_81595788689580.

Attention-gated GRU encoder: per time step
    w1 = h @ attn1_W.T + attn1_b
    w2 = x_t @ attn2_W.T + attn2_b
    v  = tanh(w1 + w2) @ attn3_W.T + attn3_b
    alpha = softmax(v, axis=feature)
    wx = x_t * alpha
    GRU cell (r, z, n) -> h_new
Output: [B, T, H] hidden states.

Strategy (8 NeuronCores, data-parallel over batch):
  - batch 4096 -> 512 rows per core; all weights replicated.
  - everything stored TRANSPOSED on chip: features on partitions, batch on
    the free dim. Every matmul is weights-stationary with batch as the
    moving dim; fp16 matmuls (weight loads hide under the 256-col streams,
    which measured faster than fp8 DoubleRow whose 256-row loads serialize).
  - feature dim I=320 zero-padded to 384 = 3x128 partition blocks.
  - attention biases ride the pad rows for free: x row 320 is constant 1.0,
    so stationary row 320 of attn2 carries (attn1_b+attn2_b) into the u
    psum, and w2[320,320]=8 makes u[320]=tanh(8)==1.0 in fp16, which then
    carries (attn3_b - 2) through stationary row 320 of attn3 into the v
    psum (pad columns get -1e4 so exp() of pad rows is exactly 0). This
    collapses the u and ev activations to ONE ACT instruction each
    (per-instruction ACT overhead is ~280ns, the dominant ACT cost).
  - softmax denominator: two DVE adds fold the three ev planes, then a
    single all-ones matmul broadcasts the per-column sum to all 128
    partitions (1 matmul instead of 3); max-subtraction is skipped
    (attn3_b carries a -2 shift so fp16 exp stays in range).
  - sigmoid is computed as 0.5*tanh(x/2)+0.5 so every ACT op uses the
    exp_and_others table set -- avoids ~2.7us ACT table swaps per step.
  - GRU blend uses h' = n + z*(h-n) (one DVE op fewer than the
    (1-z)*n + z*h form).
  - the 512-row batch runs as 2 independent chunks of 256 so the two
    recurrences pipeline against each other across engines.
"""

import numpy as np

B, T, I, H = 4096, 24, 320, 256
NCORES = 8
BS = B // NCORES          # 512 rows per core
IP = 384                  # I padded to 3*128
KI = IP // 128            # 3 feature blocks
KH = H // 128             # 2 hidden blocks
G = 3 * H                 # 768 gate rows
NCHUNK = 2
CB = BS // NCHUNK         # 256 batch columns per chunk

_STATE = {}


def _build(t_steps=T):
    import concourse.bass as bass
    import concourse.tile as tile
    from concourse import bacc, mybir

    f32 = mybir.dt.float32
    F16 = mybir.dt.float16
    AF = mybir.ActivationFunctionType
    OP = mybir.AluOpType

    nc = bacc.Bacc("TRN2", target_bir_lowering=False, debug=False,
                   num_devices=NCORES)

    xT = nc.dram_tensor("xT", [t_steps, 128, KI, BS], F16,
                        kind="ExternalInput").ap()
    h0T = nc.dram_tensor("h0T", [128, KH, BS], F16, kind="ExternalInput").ap()
    wat1 = nc.dram_tensor("wat1", [128, KH, IP], F16, kind="ExternalInput").ap()
    wat2 = nc.dram_tensor("wat2", [128, KI, IP], F16, kind="ExternalInput").ap()
    wat3 = nc.dram_tensor("wat3", [128, KI, IP], F16, kind="ExternalInput").ap()
    wih = nc.dram_tensor("wih", [128, KI, G], F16, kind="ExternalInput").ap()
    whh = nc.dram_tensor("whh", [128, KH, G], F16, kind="ExternalInput").ap()
    onesw = nc.dram_tensor("onesw", [128, 128], F16, kind="ExternalInput").ap()
    # rz bias pre-halved for the tanh-based sigmoid
    bias_rzh_d = nc.dram_tensor("bias_rzh", [128, 4], f32,
                                kind="ExternalInput").ap()
    bias_hn_d = nc.dram_tensor("bias_hn", [128, 2], f32, kind="ExternalInput").ap()
    bias_in_d = nc.dram_tensor("bias_in", [128, 2], f32, kind="ExternalInput").ap()
    outT = nc.dram_tensor("outT", [t_steps, 128, KH, BS], F16,
                          kind="ExternalOutput").ap()

    with tile.TileContext(nc) as tc:
        with tc.tile_pool(name="const", bufs=1) as cp, \
             tc.tile_pool(name="xs", bufs=1) as xp, \
             tc.tile_pool(name="hs", bufs=1) as hp, \
             tc.tile_pool(name="wk", bufs=1) as wp, \
             tc.tile_pool(name="ps", bufs=1, space="PSUM") as pp:

            w1t = cp.tile([128, KH, IP], F16)
            w2t = cp.tile([128, KI, IP], F16)
            w3t = cp.tile([128, KI, IP], F16)
            wiht = cp.tile([128, KI, G], F16)
            whht = cp.tile([128, KH, G], F16)
            onest = cp.tile([128, 128], F16)
            brzh = cp.tile([128, 4], f32)
            bhn = cp.tile([128, 2], f32)
            bin_ = cp.tile([128, 2], f32)
            # h0 + step-0 x first (they gate the first matmuls), then
            # weights ordered by first use, alternating the two HWDGE rings
            hcur = []
            for ci in range(NCHUNK):
                hc = hp.tile([128, KH, CB], F16, name=f"h_{ci}",
                             tag=f"h{ci}", bufs=2)
                nc.scalar.dma_start(
                    out=hc[:], in_=h0T[:, :, ci * CB:(ci + 1) * CB])
                hcur.append(hc)
            x_pre = xp.tile([128, KI, BS], F16, name="x_pre", tag="x", bufs=4)
            nc.sync.dma_start(out=x_pre[:], in_=xT[0])
            for i, (dst, src) in enumerate([
                    (w2t, wat2), (w1t, wat1),
                    (w3t, wat3), (whht, whh),
                    (wiht, wih), (onest, onesw),
                    (brzh, bias_rzh_d), (bhn, bias_hn_d),
                    (bin_, bias_in_d)]):
                eng = nc.sync if i % 2 == 0 else nc.scalar
                eng.dma_start(out=dst[:], in_=src)

            def ms(m):
                return slice(m * 128, (m + 1) * 128)

            for t in range(t_steps):
                if t == 0:
                    x_t = x_pre
                else:
                    x_t = xp.tile([128, KI, BS], F16, name=f"x_{t}",
                                  tag="x", bufs=4)
                    nc.sync.dma_start(out=x_t[:], in_=xT[t])

                st = [{} for _ in range(NCHUNK)]

                # ---- phase 1: attention stage 1 (u = tanh(w1+w2+b)) ----
                for ci in range(NCHUNK):
                    cs = slice(ci * CB, (ci + 1) * CB)
                    h = hcur[ci]
                    ps_u = pp.tile([128, KI, CB], f32, name=f"psu_{t}_{ci}",
                                   tag="aps", bufs=2)
                    for m in range(KI):
                        for k in range(KI):
                            nc.tensor.matmul(
                                ps_u[:, m, :], w2t[:, k, ms(m)],
                                x_t[:, k, cs], start=(k == 0), stop=False)
                        for k in range(KH):
                            nc.tensor.matmul(
                                ps_u[:, m, :], w1t[:, k, ms(m)],
                                h[:, k, :], start=False, stop=(k == KH - 1))
                    u = wp.tile([128, KI, CB], F16, name=f"u_{t}_{ci}",
                                tag="u", bufs=3)
                    # bias rode the x row-320 ones-row; one ACT, no bias
                    nc.scalar.activation(u[:], ps_u[:], AF.Tanh)
                    st[ci].update(u=u)

                # ---- phase 2: v, softmax, wx ----
                for ci in range(NCHUNK):
                    cs = slice(ci * CB, (ci + 1) * CB)
                    u = st[ci]["u"]
                    ps_v = pp.tile([128, KI, CB], f32, name=f"psv_{t}_{ci}",
                                   tag="aps", bufs=2)
                    for m in range(KI):
                        for k in range(KI):
                            nc.tensor.matmul(
                                ps_v[:, m, :], w3t[:, k, ms(m)],
                                u[:, k, :], start=(k == 0), stop=(k == KI - 1))
                    ev = wp.tile([128, KI, CB], F16, name=f"ev_{t}_{ci}",
                                 tag="ev", bufs=3)
                    nc.scalar.activation(ev[:], ps_v[:], AF.Exp)
                    evs = wp.tile([128, CB], F16, name=f"evs_{t}_{ci}",
                                  tag="evs", bufs=3)
                    nc.vector.tensor_add(evs[:], ev[:, 0, :], ev[:, 1, :])
                    evs2 = wp.tile([128, CB], F16, name=f"evs2_{t}_{ci}",
                                   tag="evs2", bufs=3)
                    nc.vector.tensor_add(evs2[:], evs[:], ev[:, 2, :])
                    ps_den = pp.tile([128, CB], f32, name=f"psden_{t}_{ci}",
                                     tag="dps", bufs=1)
                    nc.tensor.matmul(ps_den[:], onest[:], evs2[:],
                                     start=True, stop=True)
                    rinv = wp.tile([128, CB], f32, name=f"rinv_{t}_{ci}",
                                   tag="rinv", bufs=3)
                    nc.vector.reciprocal_approx_fast(rinv[:], ps_den[:])
                    rinv16 = wp.tile([128, CB], F16, name=f"rinv16_{t}_{ci}",
                                     tag="rinv16", bufs=3)
                    nc.vector.tensor_copy(rinv16[:], rinv[:])
                    wx = wp.tile([128, KI, CB], F16, name=f"wx_{t}_{ci}",
                                 tag="wx", bufs=3)
                    nc.vector.tensor_mul(wx[:], x_t[:, :, cs], ev[:])
                    _r = rinv16[:]
                    nc.vector.tensor_mul(wx[:, 0, :], wx[:, 0, :], _r)
                    rrep = bass.AP(tensor=_r.tensor, offset=_r.offset,
                                   ap=[_r.ap[0], [0, KI - 1], _r.ap[1]])
                    nc.vector.tensor_mul(wx[:, 1:KI, :], wx[:, 1:KI, :],
                                         rrep)
                    st[ci].update(wx=wx)

                # ---- phase 3: gate matmuls + GRU tail ----
                for ci in range(NCHUNK):
                    cs = slice(ci * CB, (ci + 1) * CB)
                    h = hcur[ci]
                    wx = st[ci]["wx"]
                    ps_hn = pp.tile([128, 2, CB], f32, name=f"pshn_{t}_{ci}",
                                    tag="gps", bufs=3)
                    for m in range(2):
                        for k in range(KH):
                            nc.tensor.matmul(
                                ps_hn[:, m, :], whht[:, k, ms(4 + m)],
                                h[:, k, :], start=(k == 0), stop=(k == KH - 1))
                    ps_r = pp.tile([128, 2, CB], f32, name=f"psr_{t}_{ci}",
                                   tag="gps", bufs=3)
                    ps_z = pp.tile([128, 2, CB], f32, name=f"psz_{t}_{ci}",
                                   tag="gps", bufs=3)
                    # h-only whh matmuls of the m0 slices first (r and z are
                    # different banks, so both groups may be open at once):
                    # they keep the in-order PE stream fed while wx lands
                    for mm_t, base in ((ps_r, 0), (ps_z, 2)):
                        for k in range(KH):
                            nc.tensor.matmul(
                                mm_t[:, 0, :], whht[:, k, ms(base)],
                                h[:, k, :], start=(k == 0), stop=False)
                    for mm_t, base in ((ps_r, 0), (ps_z, 2)):
                        for k in range(KI):
                            nc.tensor.matmul(
                                mm_t[:, 0, :], wiht[:, k, ms(base)],
                                wx[:, k, :], start=False, stop=(k == KI - 1))
                        for k in range(KH):
                            nc.tensor.matmul(
                                mm_t[:, 1, :], whht[:, k, ms(base + 1)],
                                h[:, k, :], start=(k == 0), stop=False)
                        for k in range(KI):
                            nc.tensor.matmul(
                                mm_t[:, 1, :], wiht[:, k, ms(base + 1)],
                                wx[:, k, :], start=False, stop=(k == KI - 1))
                    ps_in = pp.tile([128, 2, CB], f32, name=f"psin_{t}_{ci}",
                                    tag="gps", bufs=3)
                    for m in range(2):
                        for k in range(KI):
                            nc.tensor.matmul(
                                ps_in[:, m, :], wiht[:, k, ms(4 + m)],
                                wx[:, k, :], start=(k == 0), stop=(k == KI - 1))

                    g = wp.tile([128, 4, CB], F16, name=f"g_{t}_{ci}",
                                tag="g", bufs=3)
                    for m in range(4):
                        src_ps = ps_r if m < 2 else ps_z
                        nc.scalar.activation(g[:, m, :], src_ps[:, m % 2, :],
                                             AF.Tanh, bias=brzh[:, m:m + 1],
                                             scale=0.5)
                    t1h = wp.tile([128, 2, CB], F16, name=f"t1h_{t}_{ci}",
                                  tag="t1h", bufs=3)
                    for m in range(2):
                        nc.vector.tensor_scalar(
                            out=t1h[:, m, :], in0=ps_hn[:, m, :],
                            scalar1=bhn[:, m:m + 1], scalar2=0.5,
                            op0=OP.add, op1=OP.mult)
                    # p = (i_n + b_in) + t1h is g-independent: compute it
                    # early so only two fp16 DVE ops trail the gate ACT
                    p_ = wp.tile([128, 2, CB], F16, name=f"p_{t}_{ci}",
                                 tag="p", bufs=3)
                    for m in range(2):
                        nc.vector.scalar_tensor_tensor(
                            p_[:, m, :], ps_in[:, m, :], bin_[:, m:m + 1],
                            t1h[:, m, :], OP.add, OP.add)
                    t0h = wp.tile([128, 2, CB], F16, name=f"t0h_{t}_{ci}",
                                  tag="t0h", bufs=3)
                    nc.vector.tensor_mul(t0h[:], t1h[:], g[:, 0:2, :])
                    s2 = wp.tile([128, 2, CB], F16, name=f"s2_{t}_{ci}",
                                 tag="s2", bufs=3)
                    nc.vector.tensor_add(s2[:], t0h[:], p_[:])
                    n = wp.tile([128, 2, CB], F16, name=f"n_{t}_{ci}",
                                tag="n", bufs=3)
                    nc.scalar.activation(n[:], s2[:], AF.Tanh)

                    # h' = n + z*(h - n)   (z = 0.5*g_z + 0.5)
                    zz = wp.tile([128, 2, CB], F16, name=f"zz_{t}_{ci}",
                                 tag="zz", bufs=3)
                    nc.vector.tensor_scalar(
                        out=zz[:], in0=g[:, 2:4, :], scalar1=0.5, scalar2=0.5,
                        op0=OP.mult, op1=OP.add)
                    dhn = wp.tile([128, 2, CB], F16, name=f"dhn_{t}_{ci}",
                                  tag="dhn", bufs=3)
                    nc.vector.tensor_sub(dhn[:], h[:], n[:])
                    zd = wp.tile([128, 2, CB], F16, name=f"zd_{t}_{ci}",
                                 tag="zd", bufs=3)
                    nc.vector.tensor_mul(zd[:], zz[:], dhn[:])
                    h_new = hp.tile([128, KH, CB], F16, name=f"hn_{t}_{ci}",
                                    tag=f"h{ci}", bufs=2)
                    nc.vector.tensor_add(h_new[:], n[:], zd[:])
                    hcur[ci] = h_new

                    nc.sync.dma_start(out=outT[t][:, :, cs], in_=h_new[:])

    nc.compile()
    return nc


# ---------------- host-side data prep ----------------

def _prep_core_inputs(x, h0, attn1_W, attn1_b, attn2_W, attn2_b, attn3_W,
                      attn3_b, W_ih, b_ih, W_hh, b_hh, t_steps=T):
    f4 = np.float32
    f2 = np.float16
    x = np.asarray(x, f4)
    h0 = np.asarray(h0, f4)

    A1 = np.asarray(attn1_W, f4)                       # [I, H]
    w1 = np.zeros((H, IP), f4)
    w1[:, :I] = A1.T                                   # lhsT[hh, ii]
    wat1 = np.ascontiguousarray(
        w1.reshape(KH, 128, IP).transpose(1, 0, 2)).astype(f2)

    A2 = np.asarray(attn2_W, f4)                       # [I, I] (out, in)
    w2 = np.zeros((IP, IP), f4)
    w2[:I, :I] = A2.T                                  # lhsT[in, out]
    # bias rides pad row 320 against the constant-1.0 row of x;
    # w2[320,320]=8 makes u[320]=tanh(8)==1.0 in fp16 (the ev bias hook)
    w2[I, :I] = np.asarray(attn1_b, f4) + np.asarray(attn2_b, f4)
    w2[I, I] = 8.0
    wat2 = np.ascontiguousarray(
        w2.reshape(KI, 128, IP).transpose(1, 0, 2)).astype(f2)

    A3 = np.asarray(attn3_W, f4)
    w3 = np.zeros((IP, IP), f4)
    w3[:I, :I] = A3.T
    # v bias (with -2 exp shift) rides u's row 320 (==1.0); pad output
    # columns get -1e4 so exp() is exactly 0 there
    w3[I, :I] = np.asarray(attn3_b, f4) - 2.0
    w3[I, I:] = -1e4
    wat3 = np.ascontiguousarray(
        w3.reshape(KI, 128, IP).transpose(1, 0, 2)).astype(f2)

    Wi = np.asarray(W_ih, f4)                          # [G, I]
    wi = np.zeros((IP, G), f4)
    wi[:I, :] = Wi.T
    wih = np.ascontiguousarray(
        wi.reshape(KI, 128, G).transpose(1, 0, 2)).astype(f2)

    Wh = np.asarray(W_hh, f4)                          # [G, H]
    whh = np.ascontiguousarray(
        Wh.T.reshape(KH, 128, G).transpose(1, 0, 2)).astype(f2)

    onesw = np.ones((128, 128), f2)

    brz = (np.asarray(b_ih, f4) + np.asarray(b_hh, f4))[:2 * H] * 0.5
    bias_rzh = np.ascontiguousarray(brz.reshape(4, 128).T)
    bias_hn = np.ascontiguousarray(
        np.asarray(b_hh, f4)[2 * H:].reshape(2, 128).T)
    bias_in = np.ascontiguousarray(
        np.asarray(b_ih, f4)[2 * H:].reshape(2, 128).T)

    x16 = x[:, :t_steps, :].astype(f2)
    xpad = np.pad(x16, ((0, 0), (0, 0), (0, IP - I)))
    xpad[:, :, I] = 1.0                                # the bias ones-row
    # [NC, BS, T, KI, 128] -> [NC, T, 128, KI, BS]
    xr = xpad.reshape(NCORES, BS, t_steps, KI, 128).transpose(0, 2, 4, 3, 1)
    h0r = h0.astype(f2).reshape(NCORES, BS, KH, 128).transpose(0, 3, 2, 1)

    shared = dict(wat1=wat1, wat2=wat2, wat3=wat3, wih=wih, whh=whh,
                  onesw=onesw, bias_rzh=bias_rzh, bias_hn=bias_hn,
                  bias_in=bias_in)
    in_maps = []
    for c in range(NCORES):
        m = dict(shared)
        m["xT"] = np.ascontiguousarray(xr[c])
        m["h0T"] = np.ascontiguousarray(h0r[c])
        in_maps.append(m)
    return in_maps


def _gather(results, t_steps=T):
    outs = []
    for c in range(NCORES):
        o = np.asarray(results[c]["outT"], np.float32)
        outs.append(o.transpose(3, 0, 2, 1).reshape(BS, t_steps, H))
    return np.ascontiguousarray(np.concatenate(outs, axis=0))


def _get_nc(t_steps=T):
    key = ("nc", t_steps)
    if key not in _STATE:
        _STATE[key] = _build(t_steps)
    return _STATE[key]


def run(inputs, trace=False, t_steps=T):
    from concourse.bass_utils import run_bass_kernel_spmd
    nc = _get_nc(t_steps)
    in_maps = _prep_core_inputs(t_steps=t_steps, **inputs)
    res = run_bass_kernel_spmd(nc, in_maps, list(range(NCORES)), trace=trace)
    return _gather(res.results, t_steps), res


def kernel(**inputs):
    out, _ = run(inputs, trace=False)
    return out


# revision 20
# speedup vs baseline: 1.3912x; 1.1249x over previous
"""Trainium2 Bass kernel for nn_Encoder_81595788689580.

Attention-gated GRU encoder: per time step
    w1 = h @ attn1_W.T + attn1_b
    w2 = x_t @ attn2_W.T + attn2_b
    v  = tanh(w1 + w2) @ attn3_W.T + attn3_b
    alpha = softmax(v, axis=feature)
    wx = x_t * alpha
    GRU cell (r, z, n) -> h_new
Output: [B, T, H] hidden states.

Strategy (8 NeuronCores, data-parallel over batch):
  - batch 4096 -> 512 rows per core; all weights replicated.
  - everything stored TRANSPOSED on chip: features on partitions, batch on
    the free dim. Every matmul is weights-stationary with batch as the
    moving dim, biases become per-partition ACT bias vectors, and no
    transposes are ever needed on device (host pre-/post-transposes).
  - feature dim I=320 zero-padded to 384 = 3x128 partition blocks; padded
    attn3_b rows are -1e4 so exp() of pad rows is exactly 0 and the
    softmax denominator is unaffected.
  - softmax over features is a partition reduction: an all-ones stationary
    matmul broadcasts the per-column denominator into all 128 partitions
    of one PSUM tile; max-subtraction is skipped (|v| <= ~8 in practice,
    exp stays finite, softmax is shift-invariant).
  - sigmoid is computed as 0.5*tanh(x/2)+0.5 so every ACT op uses the
    exp_and_others table set -- avoids ~2.7us ACT table swaps per step.
  - matmuls in fp16 (1 PE cycle/row, fast weight loads) with fp32 PSUM
    accumulation; attn3_b carries a -2 shift for fp16 exp range. DT="f32r" switches
    to float32r matmuls (~10x lower error, ~1.7x slower weight loads).
  - the 512-row batch runs as 2 independent chunks of 256 so the two
    recurrences pipeline against each other across engines.
"""

import numpy as np

B, T, I, H = 4096, 24, 320, 256
NCORES = 8
BS = B // NCORES          # 512 rows per core
IP = 384                  # I padded to 3*128
KI = IP // 128            # 3 feature blocks
KH = H // 128             # 2 hidden blocks
G = 3 * H                 # 768 gate rows
NCHUNK = 2
CB = BS // NCHUNK         # 256 batch columns per chunk

DT = "f16"                # "f16" | "f32r"

_STATE = {}


def _np_dt(mdt):
    from concourse import mybir
    return mybir.dt.np(mdt)


def _dts():
    from concourse import mybir
    if DT == "f16":
        return mybir.dt.float16, mybir.dt.float16
    return mybir.dt.float32r, mybir.dt.float32r


def _build(t_steps=T):
    import concourse.bass as bass
    import concourse.tile as tile
    from concourse import bacc, mybir

    f32 = mybir.dt.float32
    MMD, EVD = _dts()
    AF = mybir.ActivationFunctionType
    OP = mybir.AluOpType

    nc = bacc.Bacc("TRN2", target_bir_lowering=False, debug=False,
                   num_devices=NCORES)

    xT = nc.dram_tensor("xT", [t_steps, 128, KI, BS], MMD,
                        kind="ExternalInput").ap()
    h0T = nc.dram_tensor("h0T", [128, KH, BS], MMD, kind="ExternalInput").ap()
    wat1 = nc.dram_tensor("wat1", [128, KH, IP], MMD, kind="ExternalInput").ap()
    wat2 = nc.dram_tensor("wat2", [128, KI, IP], MMD, kind="ExternalInput").ap()
    wat3 = nc.dram_tensor("wat3", [128, KI, IP], MMD, kind="ExternalInput").ap()
    wih = nc.dram_tensor("wih", [128, KI, G], MMD, kind="ExternalInput").ap()
    whh = nc.dram_tensor("whh", [128, KH, G], MMD, kind="ExternalInput").ap()
    onesw = nc.dram_tensor("onesw", [128, 128], EVD, kind="ExternalInput").ap()
    bias_u_d = nc.dram_tensor("bias_u", [128, KI], f32, kind="ExternalInput").ap()
    bias_v_d = nc.dram_tensor("bias_v", [128, KI], f32, kind="ExternalInput").ap()
    # rz bias pre-halved for the tanh-based sigmoid
    bias_rzh_d = nc.dram_tensor("bias_rzh", [128, 4], f32,
                                kind="ExternalInput").ap()
    bias_hn_d = nc.dram_tensor("bias_hn", [128, 2], f32, kind="ExternalInput").ap()
    bias_in_d = nc.dram_tensor("bias_in", [128, 2], f32, kind="ExternalInput").ap()
    outT = nc.dram_tensor("outT", [t_steps, 128, KH, BS], MMD,
                          kind="ExternalOutput").ap()

    def fv(ap):
        # readable view for DVE of matmul-dtype tiles
        if DT == "f32r":
            return ap.bitcast(f32)
        return ap

    with tile.TileContext(nc) as tc:
        with tc.tile_pool(name="const", bufs=1) as cp, \
             tc.tile_pool(name="xs", bufs=1) as xp, \
             tc.tile_pool(name="hs", bufs=1) as hp, \
             tc.tile_pool(name="wk", bufs=1) as wp, \
             tc.tile_pool(name="ps", bufs=1, space="PSUM") as pp:

            w1t = cp.tile([128, KH, IP], MMD)
            w2t = cp.tile([128, KI, IP], MMD)
            w3t = cp.tile([128, KI, IP], MMD)
            wiht = cp.tile([128, KI, G], MMD)
            whht = cp.tile([128, KH, G], MMD)
            onest = cp.tile([128, 128], EVD)
            bu = cp.tile([128, KI], f32)
            bv = cp.tile([128, KI], f32)
            brzh = cp.tile([128, 4], f32)
            bhn = cp.tile([128, 2], f32)
            bin_ = cp.tile([128, 2], f32)
            # h0 + step-0 x first (they gate the first matmuls), then
            # weights ordered by first use, alternating the two HWDGE rings
            hcur = []
            for ci in range(NCHUNK):
                hc = hp.tile([128, KH, CB], MMD, name=f"h_{ci}",
                             tag=f"h{ci}", bufs=2)
                nc.scalar.dma_start(
                    out=hc[:], in_=h0T[:, :, ci * CB:(ci + 1) * CB])
                hcur.append(hc)
            x_pre = xp.tile([128, KI, BS], MMD, name="x_pre", tag="x", bufs=4)
            nc.sync.dma_start(out=x_pre[:], in_=xT[0])
            for i, (dst, src) in enumerate([
                    (w2t, wat2), (w1t, wat1), (bu, bias_u_d),
                    (w3t, wat3), (wiht, wih), (bv, bias_v_d),
                    (whht, whh), (onest, onesw),
                    (brzh, bias_rzh_d), (bhn, bias_hn_d),
                    (bin_, bias_in_d)]):
                eng = nc.sync if i % 2 == 0 else nc.scalar
                eng.dma_start(out=dst[:], in_=src)

            def ms(m):
                return slice(m * 128, (m + 1) * 128)

            for t in range(t_steps):
                if t == 0:
                    x_t = x_pre
                else:
                    x_t = xp.tile([128, KI, BS], MMD, name=f"x_{t}",
                                  tag="x", bufs=4)
                    nc.sync.dma_start(out=x_t[:], in_=xT[t])

                st = [{} for _ in range(NCHUNK)]

                # ---- phase 1: h-gate matmuls + attention stage 1 ----
                for ci in range(NCHUNK):
                    cs = slice(ci * CB, (ci + 1) * CB)
                    h = hcur[ci]
                    ps_u = [pp.tile([128, CB], f32,
                                    name=f"psu{m}_{t}_{ci}", tag="aps",
                                    bufs=5) for m in range(KI)]
                    for m in range(KI):
                        for k in range(KI):
                            nc.tensor.matmul(
                                ps_u[m][:], w2t[:, k, ms(m)],
                                x_t[:, k, cs], start=(k == 0), stop=False)
                        for k in range(KH):
                            nc.tensor.matmul(
                                ps_u[m][:], w1t[:, k, ms(m)],
                                h[:, k, :], start=False, stop=(k == KH - 1))
                    u = wp.tile([128, KI, CB], MMD, name=f"u_{t}_{ci}",
                                tag="u", bufs=3)
                    for m in range(KI):
                        nc.scalar.activation(u[:, m, :], ps_u[m][:],
                                             AF.Tanh, bias=bu[:, m:m + 1])
                    st[ci].update(u=u)

                # ---- phase 2: v, softmax, wx ----
                for ci in range(NCHUNK):
                    cs = slice(ci * CB, (ci + 1) * CB)
                    u = st[ci]["u"]
                    ps_v = [pp.tile([128, CB], f32,
                                    name=f"psv{m}_{t}_{ci}", tag="aps",
                                    bufs=5) for m in range(KI)]
                    for m in range(KI):
                        for k in range(KI):
                            nc.tensor.matmul(
                                ps_v[m][:], w3t[:, k, ms(m)],
                                u[:, k, :], start=(k == 0), stop=(k == KI - 1))
                    ev = wp.tile([128, KI, CB], EVD, name=f"ev_{t}_{ci}",
                                 tag="ev", bufs=3)
                    for m in range(KI):
                        nc.scalar.activation(ev[:, m, :], ps_v[m][:],
                                             AF.Exp, bias=bv[:, m:m + 1])
                    ps_den = pp.tile([128, CB], f32, name=f"psden_{t}_{ci}",
                                     tag="aps", bufs=5)
                    for k in range(KI):
                        nc.tensor.matmul(ps_den[:], onest[:], ev[:, k, :],
                                         start=(k == 0), stop=(k == KI - 1))
                    rinv = wp.tile([128, CB], f32, name=f"rinv_{t}_{ci}",
                                   tag="rinv", bufs=3)
                    nc.vector.reciprocal_approx_fast(rinv[:], ps_den[:])
                    rinv16 = wp.tile([128, CB], MMD, name=f"rinv16_{t}_{ci}",
                                     tag="rinv16", bufs=3)
                    nc.vector.tensor_copy(rinv16[:], rinv[:])
                    wx = wp.tile([128, KI, CB], MMD, name=f"wx_{t}_{ci}",
                                 tag="wx", bufs=3)
                    nc.vector.tensor_mul(wx[:], fv(x_t[:, :, cs]), fv(ev[:]))
                    _r = rinv16[:]
                    nc.vector.tensor_mul(wx[:, 0, :], fv(wx[:, 0, :]), _r)
                    rrep = bass.AP(tensor=_r.tensor, offset=_r.offset,
                                   ap=[_r.ap[0], [0, KI - 1], _r.ap[1]])
                    nc.vector.tensor_mul(wx[:, 1:KI, :], fv(wx[:, 1:KI, :]),
                                         rrep)
                    st[ci].update(wx=wx)

                # ---- phase 3: gate matmuls + GRU tail ----
                for ci in range(NCHUNK):
                    cs = slice(ci * CB, (ci + 1) * CB)
                    h = hcur[ci]
                    wx = st[ci]["wx"]
                    ps_hn = pp.tile([128, 2, CB], f32, name=f"pshn_{t}_{ci}",
                                    tag="gps", bufs=3)
                    for m in range(2):
                        for k in range(KH):
                            nc.tensor.matmul(
                                ps_hn[:, m, :], whht[:, k, ms(4 + m)],
                                h[:, k, :], start=(k == 0), stop=(k == KH - 1))
                    ps_r = pp.tile([128, 2, CB], f32, name=f"psr_{t}_{ci}",
                                   tag="gps", bufs=3)
                    ps_z = pp.tile([128, 2, CB], f32, name=f"psz_{t}_{ci}",
                                   tag="gps", bufs=3)
                    # h-only whh matmuls of the m0 slices first (r and z are
                    # different banks, so both groups may be open at once):
                    # they keep the in-order PE stream fed while wx lands
                    for mm_t, base in ((ps_r, 0), (ps_z, 2)):
                        for k in range(KH):
                            nc.tensor.matmul(
                                mm_t[:, 0, :], whht[:, k, ms(base)],
                                h[:, k, :], start=(k == 0), stop=False)
                    for mm_t, base in ((ps_r, 0), (ps_z, 2)):
                        for k in range(KI):
                            nc.tensor.matmul(
                                mm_t[:, 0, :], wiht[:, k, ms(base)],
                                wx[:, k, :], start=False, stop=(k == KI - 1))
                        for k in range(KH):
                            nc.tensor.matmul(
                                mm_t[:, 1, :], whht[:, k, ms(base + 1)],
                                h[:, k, :], start=(k == 0), stop=False)
                        for k in range(KI):
                            nc.tensor.matmul(
                                mm_t[:, 1, :], wiht[:, k, ms(base + 1)],
                                wx[:, k, :], start=False, stop=(k == KI - 1))
                    ps_in = pp.tile([128, 2, CB], f32, name=f"psin_{t}_{ci}",
                                    tag="gps", bufs=3)
                    for m in range(2):
                        for k in range(KI):
                            nc.tensor.matmul(
                                ps_in[:, m, :], wiht[:, k, ms(4 + m)],
                                wx[:, k, :], start=(k == 0), stop=(k == KI - 1))

                    g = wp.tile([128, 4, CB], MMD, name=f"g_{t}_{ci}",
                                tag="g", bufs=3)
                    for m in range(4):
                        src_ps = ps_r if m < 2 else ps_z
                        nc.scalar.activation(g[:, m, :], src_ps[:, m % 2, :],
                                             AF.Tanh, bias=brzh[:, m:m + 1],
                                             scale=0.5)
                    t1h = wp.tile([128, 2, CB], MMD, name=f"t1h_{t}_{ci}",
                                  tag="t1h", bufs=3)
                    for m in range(2):
                        nc.vector.tensor_scalar(
                            out=t1h[:, m, :], in0=ps_hn[:, m, :],
                            scalar1=bhn[:, m:m + 1], scalar2=0.5,
                            op0=OP.add, op1=OP.mult)
                    # p = (i_n + b_in) + t1h is g-independent: compute it
                    # early so only two fp16 DVE ops trail the gate ACT
                    p_ = wp.tile([128, 2, CB], MMD, name=f"p_{t}_{ci}",
                                 tag="p", bufs=3)
                    for m in range(2):
                        nc.vector.scalar_tensor_tensor(
                            p_[:, m, :], ps_in[:, m, :], bin_[:, m:m + 1],
                            t1h[:, m, :], OP.add, OP.add)
                    t0h = wp.tile([128, 2, CB], MMD, name=f"t0h_{t}_{ci}",
                                  tag="t0h", bufs=3)
                    nc.vector.tensor_mul(t0h[:], t1h[:], g[:, 0:2, :])
                    s2 = wp.tile([128, 2, CB], MMD, name=f"s2_{t}_{ci}",
                                 tag="s2", bufs=3)
                    nc.vector.tensor_add(s2[:], t0h[:], p_[:])
                    n = wp.tile([128, 2, CB], MMD, name=f"n_{t}_{ci}",
                                tag="n", bufs=3)
                    nc.scalar.activation(n[:], s2[:], AF.Tanh)

                    zz = wp.tile([128, 2, CB], MMD, name=f"zz_{t}_{ci}",
                                 tag="zz", bufs=3)
                    nc.vector.tensor_scalar(
                        out=zz[:], in0=g[:, 2:4, :], scalar1=0.5, scalar2=0.5,
                        op0=OP.mult, op1=OP.add)
                    # h' = n + z*(h - n)
                    dhn = wp.tile([128, 2, CB], MMD, name=f"dhn_{t}_{ci}",
                                  tag="dhn", bufs=3)
                    nc.vector.tensor_sub(dhn[:], fv(h[:]), n[:])
                    zd = wp.tile([128, 2, CB], MMD, name=f"zd_{t}_{ci}",
                                 tag="zd", bufs=3)
                    nc.vector.tensor_mul(zd[:], zz[:], dhn[:])
                    h_new = hp.tile([128, KH, CB], MMD, name=f"hn_{t}_{ci}",
                                    tag=f"h{ci}", bufs=2)
                    nc.vector.tensor_add(h_new[:], n[:], zd[:])
                    hcur[ci] = h_new

                    nc.sync.dma_start(out=outT[t][:, :, cs], in_=h_new[:])

    nc.compile()
    return nc


# ---------------- host-side data prep ----------------

def _prep_core_inputs(x, h0, attn1_W, attn1_b, attn2_W, attn2_b, attn3_W,
                      attn3_b, W_ih, b_ih, W_hh, b_hh, t_steps=T):
    f4 = np.float32
    MMD, EVD = _dts()
    mnp = _np_dt(MMD)
    enp = _np_dt(EVD)
    x = np.asarray(x, f4)
    h0 = np.asarray(h0, f4)

    A1 = np.asarray(attn1_W, f4)                       # [I, H]
    w1 = np.zeros((H, IP), f4)
    w1[:, :I] = A1.T                                   # lhsT[hh, ii]
    wat1 = np.ascontiguousarray(
        w1.reshape(KH, 128, IP).transpose(1, 0, 2)).astype(mnp)

    A2 = np.asarray(attn2_W, f4)                       # [I, I] (out, in)
    w2 = np.zeros((IP, IP), f4)
    w2[:I, :I] = A2.T                                  # lhsT[in, out]
    wat2 = np.ascontiguousarray(
        w2.reshape(KI, 128, IP).transpose(1, 0, 2)).astype(mnp)

    A3 = np.asarray(attn3_W, f4)
    w3 = np.zeros((IP, IP), f4)
    w3[:I, :I] = A3.T
    wat3 = np.ascontiguousarray(
        w3.reshape(KI, 128, IP).transpose(1, 0, 2)).astype(mnp)

    Wi = np.asarray(W_ih, f4)                          # [G, I]
    wi = np.zeros((IP, G), f4)
    wi[:I, :] = Wi.T
    wih = np.ascontiguousarray(
        wi.reshape(KI, 128, G).transpose(1, 0, 2)).astype(mnp)

    Wh = np.asarray(W_hh, f4)                          # [G, H]
    whh = np.ascontiguousarray(
        Wh.T.reshape(KH, 128, G).transpose(1, 0, 2)).astype(mnp)

    onesw = np.ones((128, 128), enp)

    bu = np.zeros(IP, f4)
    bu[:I] = np.asarray(attn1_b, f4) + np.asarray(attn2_b, f4)
    bias_u = np.ascontiguousarray(bu.reshape(KI, 128).T)
    bvv = np.full(IP, -1e4, f4)
    bvv[:I] = np.asarray(attn3_b, f4) - 2.0   # shift-invariant, fp16 range
    bias_v = np.ascontiguousarray(bvv.reshape(KI, 128).T)
    brz = (np.asarray(b_ih, f4) + np.asarray(b_hh, f4))[:2 * H] * 0.5
    bias_rzh = np.ascontiguousarray(brz.reshape(4, 128).T)
    bias_hn = np.ascontiguousarray(
        np.asarray(b_hh, f4)[2 * H:].reshape(2, 128).T)
    bias_in = np.ascontiguousarray(
        np.asarray(b_ih, f4)[2 * H:].reshape(2, 128).T)

    x16 = x[:, :t_steps, :].astype(mnp)
    xpad = np.pad(x16, ((0, 0), (0, 0), (0, IP - I)))
    # [NC, BS, T, KI, 128] -> [NC, T, 128, KI, BS]
    xr = xpad.reshape(NCORES, BS, t_steps, KI, 128).transpose(0, 2, 4, 3, 1)
    h0r = h0.astype(mnp).reshape(NCORES, BS, KH, 128).transpose(0, 3, 2, 1)

    shared = dict(wat1=wat1, wat2=wat2, wat3=wat3, wih=wih, whh=whh,
                  onesw=onesw, bias_u=bias_u, bias_v=bias_v,
                  bias_rzh=bias_rzh, bias_hn=bias_hn, bias_in=bias_in)
    in_maps = []
    for c in range(NCORES):
        m = dict(shared)
        m["xT"] = np.ascontiguousarray(xr[c])
        m["h0T"] = np.ascontiguousarray(h0r[c])
        in_maps.append(m)
    return in_maps


def _gather(results, t_steps=T):
    outs = []
    for c in range(NCORES):
        o = np.asarray(results[c]["outT"], np.float32)
        outs.append(o.transpose(3, 0, 2, 1).reshape(BS, t_steps, H))
    return np.ascontiguousarray(np.concatenate(outs, axis=0))


def _get_nc(t_steps=T):
    key = ("nc", t_steps, DT)
    if key not in _STATE:
        _STATE[key] = _build(t_steps)
    return _STATE[key]


def run(inputs, trace=False, t_steps=T):
    from concourse.bass_utils import run_bass_kernel_spmd
    nc = _get_nc(t_steps)
    in_maps = _prep_core_inputs(t_steps=t_steps, **inputs)
    res = run_bass_kernel_spmd(nc, in_maps, list(range(NCORES)), trace=trace)
    return _gather(res.results, t_steps), res


def kernel(**inputs):
    out, _ = run(inputs, trace=False)
    return out



# revision 21
# speedup vs baseline: 1.4368x; 1.0328x over previous
"""Trainium2 Bass kernel for nn_Encoder_81595788689580.

Attention-gated GRU encoder: per time step
    w1 = h @ attn1_W.T + attn1_b
    w2 = x_t @ attn2_W.T + attn2_b
    v  = tanh(w1 + w2) @ attn3_W.T + attn3_b
    alpha = softmax(v, axis=feature)
    wx = x_t * alpha
    GRU cell (r, z, n) -> h_new
Output: [B, T, H] hidden states.

Strategy (8 NeuronCores, data-parallel over batch):
  - batch 4096 -> 512 rows per core; all weights replicated.
  - everything stored TRANSPOSED on chip: features on partitions, batch on
    the free dim. Every matmul is weights-stationary with batch as the
    moving dim, biases become per-partition ACT bias vectors, and no
    transposes are ever needed on device (host pre-/post-transposes).
  - feature dim I=320 zero-padded to 384 = 3x128 partition blocks; padded
    attn3_b rows are -1e4 so exp() of pad rows is exactly 0 and the
    softmax denominator is unaffected.
  - softmax over features is a partition reduction: an all-ones stationary
    matmul broadcasts the per-column denominator into all 128 partitions
    of one PSUM tile; max-subtraction is skipped (|v| <= ~8 in practice,
    exp stays finite, softmax is shift-invariant).
  - sigmoid is computed as 0.5*tanh(x/2)+0.5 so every ACT op uses the
    exp_and_others table set -- avoids ~2.7us ACT table swaps per step.
  - matmuls in fp16 (1 PE cycle/row, fast weight loads) with fp32 PSUM
    accumulation; attn3_b carries a -2 shift for fp16 exp range. DT="f32r" switches
    to float32r matmuls (~10x lower error, ~1.7x slower weight loads).
  - the 512-row batch runs as 2 independent chunks of 256 so the two
    recurrences pipeline against each other across engines.
"""

import numpy as np

B, T, I, H = 4096, 24, 320, 256
NCORES = 8
BS = B // NCORES          # 512 rows per core
IP = 384                  # I padded to 3*128
KI = IP // 128            # 3 feature blocks
KH = H // 128             # 2 hidden blocks
G = 3 * H                 # 768 gate rows
NCHUNK = 2
CB = BS // NCHUNK         # 256 batch columns per chunk

DT = "f16"                # "f16" | "f32r"

_STATE = {}


def _np_dt(mdt):
    from concourse import mybir
    return mybir.dt.np(mdt)


def _dts():
    from concourse import mybir
    if DT == "f16":
        return mybir.dt.float16, mybir.dt.float16
    return mybir.dt.float32r, mybir.dt.float32r


def _build(t_steps=T):
    import concourse.bass as bass
    import concourse.tile as tile
    from concourse import bacc, mybir

    f32 = mybir.dt.float32
    MMD, EVD = _dts()
    AF = mybir.ActivationFunctionType
    OP = mybir.AluOpType

    nc = bacc.Bacc("TRN2", target_bir_lowering=False, debug=False,
                   num_devices=NCORES)

    xT = nc.dram_tensor("xT", [t_steps, 128, KI, BS], MMD,
                        kind="ExternalInput").ap()
    h0T = nc.dram_tensor("h0T", [128, KH, BS], MMD, kind="ExternalInput").ap()
    wat1 = nc.dram_tensor("wat1", [128, KH, IP], MMD, kind="ExternalInput").ap()
    wat2 = nc.dram_tensor("wat2", [128, KI, IP], MMD, kind="ExternalInput").ap()
    wat3 = nc.dram_tensor("wat3", [128, KI, IP], MMD, kind="ExternalInput").ap()
    wih = nc.dram_tensor("wih", [128, KI, G], MMD, kind="ExternalInput").ap()
    whh = nc.dram_tensor("whh", [128, KH, G], MMD, kind="ExternalInput").ap()
    onesw = nc.dram_tensor("onesw", [128, 128], EVD, kind="ExternalInput").ap()
    bias_u_d = nc.dram_tensor("bias_u", [128, KI], f32, kind="ExternalInput").ap()
    bias_v_d = nc.dram_tensor("bias_v", [128, KI], f32, kind="ExternalInput").ap()
    # rz bias pre-halved for the tanh-based sigmoid
    bias_rzh_d = nc.dram_tensor("bias_rzh", [128, 4], f32,
                                kind="ExternalInput").ap()
    bias_hn_d = nc.dram_tensor("bias_hn", [128, 2], f32, kind="ExternalInput").ap()
    bias_in_d = nc.dram_tensor("bias_in", [128, 2], f32, kind="ExternalInput").ap()
    outT = nc.dram_tensor("outT", [t_steps, 128, KH, BS], MMD,
                          kind="ExternalOutput").ap()

    def fv(ap):
        # readable view for DVE of matmul-dtype tiles
        if DT == "f32r":
            return ap.bitcast(f32)
        return ap

    with tile.TileContext(nc) as tc:
        with tc.tile_pool(name="const", bufs=1) as cp, \
             tc.tile_pool(name="xs", bufs=1) as xp, \
             tc.tile_pool(name="hs", bufs=1) as hp, \
             tc.tile_pool(name="wk", bufs=1) as wp, \
             tc.tile_pool(name="ps", bufs=1, space="PSUM") as pp:

            w1t = cp.tile([128, KH, IP], MMD)
            w2t = cp.tile([128, KI, IP], MMD)
            w3t = cp.tile([128, KI, IP], MMD)
            wiht = cp.tile([128, KI, G], MMD)
            whht = cp.tile([128, KH, G], MMD)
            onest = cp.tile([128, 128], EVD)
            bu = cp.tile([128, KI], f32)
            bv = cp.tile([128, KI], f32)
            brzh = cp.tile([128, 4], f32)
            bhn = cp.tile([128, 2], f32)
            bin_ = cp.tile([128, 2], f32)
            # h0 + step-0 x first (they gate the first matmuls), then
            # weights ordered by first use, alternating the two HWDGE rings
            hcur = []
            for ci in range(NCHUNK):
                hc = hp.tile([128, KH, CB], MMD, name=f"h_{ci}",
                             tag=f"h{ci}", bufs=2)
                nc.scalar.dma_start(
                    out=hc[:], in_=h0T[:, :, ci * CB:(ci + 1) * CB])
                hcur.append(hc)
            x_pre = xp.tile([128, KI, BS], MMD, name="x_pre", tag="x", bufs=4)
            nc.sync.dma_start(out=x_pre[:], in_=xT[0])
            for i, (dst, src) in enumerate([
                    (w2t, wat2), (w1t, wat1), (bu, bias_u_d),
                    (w3t, wat3), (bv, bias_v_d), (onest, onesw),
                    (whht, whh), (wiht, wih),
                    (brzh, bias_rzh_d), (bhn, bias_hn_d),
                    (bin_, bias_in_d)]):
                eng = nc.sync if i % 2 == 0 else nc.scalar
                eng.dma_start(out=dst[:], in_=src)

            def ms(m):
                return slice(m * 128, (m + 1) * 128)

            for t in range(t_steps):
                if t == 0:
                    x_t = x_pre
                else:
                    x_t = xp.tile([128, KI, BS], MMD, name=f"x_{t}",
                                  tag="x", bufs=4)
                    nc.sync.dma_start(out=x_t[:], in_=xT[t])

                st = [{} for _ in range(NCHUNK)]

                # ---- phase 1: h-gate matmuls + attention stage 1 ----
                for ci in range(NCHUNK):
                    cs = slice(ci * CB, (ci + 1) * CB)
                    h = hcur[ci]
                    ps_u = [pp.tile([128, CB], f32,
                                    name=f"psu{m}_{t}_{ci}", tag="aps",
                                    bufs=5) for m in range(KI)]
                    for m in range(KI):
                        for k in range(KI):
                            nc.tensor.matmul(
                                ps_u[m][:], w2t[:, k, ms(m)],
                                x_t[:, k, cs], start=(k == 0), stop=False)
                        for k in range(KH):
                            nc.tensor.matmul(
                                ps_u[m][:], w1t[:, k, ms(m)],
                                h[:, k, :], start=False, stop=(k == KH - 1))
                    u = wp.tile([128, KI, CB], MMD, name=f"u_{t}_{ci}",
                                tag="u", bufs=3)
                    for m in range(KI):
                        nc.scalar.activation(u[:, m, :], ps_u[m][:],
                                             AF.Tanh, bias=bu[:, m:m + 1])
                    st[ci].update(u=u)

                # ---- phase 2: v, softmax, wx ----
                for ci in range(NCHUNK):
                    cs = slice(ci * CB, (ci + 1) * CB)
                    u = st[ci]["u"]
                    ps_v = [pp.tile([128, CB], f32,
                                    name=f"psv{m}_{t}_{ci}", tag="aps",
                                    bufs=5) for m in range(KI)]
                    for m in range(KI):
                        for k in range(KI):
                            nc.tensor.matmul(
                                ps_v[m][:], w3t[:, k, ms(m)],
                                u[:, k, :], start=(k == 0), stop=(k == KI - 1))
                    ev = wp.tile([128, KI, CB], EVD, name=f"ev_{t}_{ci}",
                                 tag="ev", bufs=3)
                    for m in range(KI):
                        nc.scalar.activation(ev[:, m, :], ps_v[m][:],
                                             AF.Exp, bias=bv[:, m:m + 1])
                    ps_den = pp.tile([128, CB], f32, name=f"psden_{t}_{ci}",
                                     tag="aps", bufs=5)
                    for k in range(KI):
                        nc.tensor.matmul(ps_den[:], onest[:], ev[:, k, :],
                                         start=(k == 0), stop=(k == KI - 1))
                    rinv = wp.tile([128, CB], f32, name=f"rinv_{t}_{ci}",
                                   tag="rinv", bufs=3)
                    nc.vector.reciprocal_approx_fast(rinv[:], ps_den[:])
                    rinv16 = wp.tile([128, CB], MMD, name=f"rinv16_{t}_{ci}",
                                     tag="rinv16", bufs=3)
                    nc.vector.tensor_copy(rinv16[:], rinv[:])
                    wx = wp.tile([128, KI, CB], MMD, name=f"wx_{t}_{ci}",
                                 tag="wx", bufs=3)
                    nc.vector.tensor_mul(wx[:], fv(x_t[:, :, cs]), fv(ev[:]))
                    _r = rinv16[:]
                    nc.vector.tensor_mul(wx[:, 0, :], fv(wx[:, 0, :]), _r)
                    rrep = bass.AP(tensor=_r.tensor, offset=_r.offset,
                                   ap=[_r.ap[0], [0, KI - 1], _r.ap[1]])
                    nc.vector.tensor_mul(wx[:, 1:KI, :], fv(wx[:, 1:KI, :]),
                                         rrep)
                    st[ci].update(wx=wx)

                # ---- phase 3: gate matmuls + GRU tail ----
                for ci in range(NCHUNK):
                    cs = slice(ci * CB, (ci + 1) * CB)
                    h = hcur[ci]
                    wx = st[ci]["wx"]
                    ps_hn = pp.tile([128, 2, CB], f32, name=f"pshn_{t}_{ci}",
                                    tag="gps", bufs=3)
                    for m in range(2):
                        for k in range(KH):
                            nc.tensor.matmul(
                                ps_hn[:, m, :], whht[:, k, ms(4 + m)],
                                h[:, k, :], start=(k == 0), stop=(k == KH - 1))
                    ps_r = pp.tile([128, 2, CB], f32, name=f"psr_{t}_{ci}",
                                   tag="gps", bufs=3)
                    ps_z = pp.tile([128, 2, CB], f32, name=f"psz_{t}_{ci}",
                                   tag="gps", bufs=3)
                    # h-only whh matmuls of the m0 slices first (r and z are
                    # different banks, so both groups may be open at once):
                    # they keep the in-order PE stream fed while wx lands
                    for mm_t, base in ((ps_r, 0), (ps_z, 2)):
                        for k in range(KH):
                            nc.tensor.matmul(
                                mm_t[:, 0, :], whht[:, k, ms(base)],
                                h[:, k, :], start=(k == 0), stop=False)
                    for mm_t, base in ((ps_r, 0), (ps_z, 2)):
                        for k in range(KI):
                            nc.tensor.matmul(
                                mm_t[:, 0, :], wiht[:, k, ms(base)],
                                wx[:, k, :], start=False, stop=(k == KI - 1))
                        for k in range(KH):
                            nc.tensor.matmul(
                                mm_t[:, 1, :], whht[:, k, ms(base + 1)],
                                h[:, k, :], start=(k == 0), stop=False)
                        for k in range(KI):
                            nc.tensor.matmul(
                                mm_t[:, 1, :], wiht[:, k, ms(base + 1)],
                                wx[:, k, :], start=False, stop=(k == KI - 1))
                    ps_in = pp.tile([128, 2, CB], f32, name=f"psin_{t}_{ci}",
                                    tag="gps", bufs=3)
                    for m in range(2):
                        for k in range(KI):
                            nc.tensor.matmul(
                                ps_in[:, m, :], wiht[:, k, ms(4 + m)],
                                wx[:, k, :], start=(k == 0), stop=(k == KI - 1))

                    g = wp.tile([128, 4, CB], MMD, name=f"g_{t}_{ci}",
                                tag="g", bufs=3)
                    for m in range(4):
                        src_ps = ps_r if m < 2 else ps_z
                        nc.scalar.activation(g[:, m, :], src_ps[:, m % 2, :],
                                             AF.Tanh, bias=brzh[:, m:m + 1],
                                             scale=0.5)
                    t1h = wp.tile([128, 2, CB], MMD, name=f"t1h_{t}_{ci}",
                                  tag="t1h", bufs=3)
                    for m in range(2):
                        nc.vector.tensor_scalar(
                            out=t1h[:, m, :], in0=ps_hn[:, m, :],
                            scalar1=bhn[:, m:m + 1], scalar2=0.5,
                            op0=OP.add, op1=OP.mult)
                    # p = (i_n + b_in) + t1h is g-independent: compute it
                    # early so only two fp16 DVE ops trail the gate ACT
                    p_ = wp.tile([128, 2, CB], MMD, name=f"p_{t}_{ci}",
                                 tag="p", bufs=3)
                    for m in range(2):
                        nc.vector.scalar_tensor_tensor(
                            p_[:, m, :], ps_in[:, m, :], bin_[:, m:m + 1],
                            t1h[:, m, :], OP.add, OP.add)
                    t0h = wp.tile([128, 2, CB], MMD, name=f"t0h_{t}_{ci}",
                                  tag="t0h", bufs=3)
                    nc.vector.tensor_mul(t0h[:], t1h[:], g[:, 0:2, :])
                    s2 = wp.tile([128, 2, CB], MMD, name=f"s2_{t}_{ci}",
                                 tag="s2", bufs=3)
                    nc.vector.tensor_add(s2[:], t0h[:], p_[:])
                    n = wp.tile([128, 2, CB], MMD, name=f"n_{t}_{ci}",
                                tag="n", bufs=3)
                    nc.scalar.activation(n[:], s2[:], AF.Tanh)

                    zz = wp.tile([128, 2, CB], MMD, name=f"zz_{t}_{ci}",
                                 tag="zz", bufs=3)
                    nc.vector.tensor_scalar(
                        out=zz[:], in0=g[:, 2:4, :], scalar1=0.5, scalar2=0.5,
                        op0=OP.mult, op1=OP.add)
                    w1z = wp.tile([128, 2, CB], MMD, name=f"w1z_{t}_{ci}",
                                  tag="w1z", bufs=3)
                    nc.vector.tensor_scalar(
                        out=w1z[:], in0=g[:, 2:4, :], scalar1=-0.5,
                        scalar2=0.5, op0=OP.mult, op1=OP.add)
                    bzh = wp.tile([128, 2, CB], MMD, name=f"bzh_{t}_{ci}",
                                  tag="bzh", bufs=3)
                    nc.vector.tensor_mul(bzh[:], zz[:], fv(h[:]))
                    a4 = wp.tile([128, 2, CB], MMD, name=f"a4_{t}_{ci}",
                                 tag="a4", bufs=3)
                    nc.vector.tensor_mul(a4[:], w1z[:], n[:])
                    h_new = hp.tile([128, KH, CB], MMD, name=f"hn_{t}_{ci}",
                                    tag=f"h{ci}", bufs=2)
                    nc.vector.tensor_add(h_new[:], a4[:], bzh[:])
                    hcur[ci] = h_new

                    nc.sync.dma_start(out=outT[t][:, :, cs], in_=h_new[:])

    nc.compile()
    return nc


# ---------------- host-side data prep ----------------

def _prep_core_inputs(x, h0, attn1_W, attn1_b, attn2_W, attn2_b, attn3_W,
                      attn3_b, W_ih, b_ih, W_hh, b_hh, t_steps=T):
    f4 = np.float32
    MMD, EVD = _dts()
    mnp = _np_dt(MMD)
    enp = _np_dt(EVD)
    x = np.asarray(x, f4)
    h0 = np.asarray(h0, f4)

    A1 = np.asarray(attn1_W, f4)                       # [I, H]
    w1 = np.zeros((H, IP), f4)
    w1[:, :I] = A1.T                                   # lhsT[hh, ii]
    wat1 = np.ascontiguousarray(
        w1.reshape(KH, 128, IP).transpose(1, 0, 2)).astype(mnp)

    A2 = np.asarray(attn2_W, f4)                       # [I, I] (out, in)
    w2 = np.zeros((IP, IP), f4)
    w2[:I, :I] = A2.T                                  # lhsT[in, out]
    wat2 = np.ascontiguousarray(
        w2.reshape(KI, 128, IP).transpose(1, 0, 2)).astype(mnp)

    A3 = np.asarray(attn3_W, f4)
    w3 = np.zeros((IP, IP), f4)
    w3[:I, :I] = A3.T
    wat3 = np.ascontiguousarray(
        w3.reshape(KI, 128, IP).transpose(1, 0, 2)).astype(mnp)

    Wi = np.asarray(W_ih, f4)                          # [G, I]
    wi = np.zeros((IP, G), f4)
    wi[:I, :] = Wi.T
    wih = np.ascontiguousarray(
        wi.reshape(KI, 128, G).transpose(1, 0, 2)).astype(mnp)

    Wh = np.asarray(W_hh, f4)                          # [G, H]
    whh = np.ascontiguousarray(
        Wh.T.reshape(KH, 128, G).transpose(1, 0, 2)).astype(mnp)

    onesw = np.ones((128, 128), enp)

    bu = np.zeros(IP, f4)
    bu[:I] = np.asarray(attn1_b, f4) + np.asarray(attn2_b, f4)
    bias_u = np.ascontiguousarray(bu.reshape(KI, 128).T)
    bvv = np.full(IP, -1e4, f4)
    bvv[:I] = np.asarray(attn3_b, f4) - 2.0   # shift-invariant, fp16 range
    bias_v = np.ascontiguousarray(bvv.reshape(KI, 128).T)
    brz = (np.asarray(b_ih, f4) + np.asarray(b_hh, f4))[:2 * H] * 0.5
    bias_rzh = np.ascontiguousarray(brz.reshape(4, 128).T)
    bias_hn = np.ascontiguousarray(
        np.asarray(b_hh, f4)[2 * H:].reshape(2, 128).T)
    bias_in = np.ascontiguousarray(
        np.asarray(b_ih, f4)[2 * H:].reshape(2, 128).T)

    x16 = x[:, :t_steps, :].astype(mnp)
    xpad = np.pad(x16, ((0, 0), (0, 0), (0, IP - I)))
    # [NC, BS, T, KI, 128] -> [NC, T, 128, KI, BS]
    xr = xpad.reshape(NCORES, BS, t_steps, KI, 128).transpose(0, 2, 4, 3, 1)
    h0r = h0.astype(mnp).reshape(NCORES, BS, KH, 128).transpose(0, 3, 2, 1)

    shared = dict(wat1=wat1, wat2=wat2, wat3=wat3, wih=wih, whh=whh,
                  onesw=onesw, bias_u=bias_u, bias_v=bias_v,
                  bias_rzh=bias_rzh, bias_hn=bias_hn, bias_in=bias_in)
    in_maps = []
    for c in range(NCORES):
        m = dict(shared)
        m["xT"] = np.ascontiguousarray(xr[c])
        m["h0T"] = np.ascontiguousarray(h0r[c])
        in_maps.append(m)
    return in_maps


def _gather(results, t_steps=T):
    outs = []
    for c in range(NCORES):
        o = np.asarray(results[c]["outT"], np.float32)
        outs.append(o.transpose(3, 0, 2, 1).reshape(BS, t_steps, H))
    return np.ascontiguousarray(np.concatenate(outs, axis=0))


def _get_nc(t_steps=T):
    key = ("nc", t_steps, DT)
    if key not in _STATE:
        _STATE[key] = _build(t_steps)
    return _STATE[key]


def run(inputs, trace=False, t_steps=T):
    from concourse.bass_utils import run_bass_kernel_spmd
    nc = _get_nc(t_steps)
    in_maps = _prep_core_inputs(t_steps=t_steps, **inputs)
    res = run_bass_kernel_spmd(nc, in_maps, list(range(NCORES)), trace=trace)
    return _gather(res.results, t_steps), res


def kernel(**inputs):
    out, _ = run(inputs, trace=False)
    return out

